# revision 1
# baseline (speedup 1.0000x reference)
"""Trainium2 Bass kernel for nn_Denoiser_73598559584966.

Full-sequence self-attention (Q=K=V, no scaling) over x: [4, 16, 16, 16, 64]
  t = x.reshape(B, 4096, 64); out = softmax(t @ t^T) @ t

Sharding: 8 cores = 4 batches x 2 query-halves. Each core: 2048 queries
vs the full 4096 keys/values of its batch. No collectives.

Device algorithm per core (scores kept transposed: [keys, queries]),
key tiles processed in pairs (ktA rows 0-63 / ktB rows 64-127 of the PE
array — the second tile's weight load hides under the first's stream):
  pass1 (fp32r, contraction 64): S += k_hi . q_hi   (exact: inputs are
         pre-rounded on host to the fp32r 11-bit grid)
  pass2 (bf16, contraction 128): S += k_lo.q_hi + k_hi.q_lo - B
         (B_i = |q_i| max_j|k_j| >= rowmax -> exp never overflows; the
          shift cancels exactly in the softmax ratio)
  P = exp(S)                      ScalarE, PSUM -> fp32r SBUF
  O^T[65, q] += (V_kt|1)^T P_kt   fp32r; row 64 = softmax denominator
Device returns O^T [65, 2048]; the host epilogue divides rows 0..63 by
row 64 and transposes while gathering shards (O(N*C) marshaling).
"""
import numpy as np

B_, D_, H_, W_, C_ = 4, 16, 16, 16, 64
NTOK = D_ * H_ * W_          # 4096 tokens per batch
NQ = NTOK // 2               # 2048 queries per core
NCORES = 8
NKT = NTOK // 128            # 32 key tiles
NPAIR = NKT // 2             # 16 packed key-tile pairs
NCH = 4                      # query chunks per core
CHW = NQ // NCH              # 512 queries per chunk
NG = 4                       # DMA groups over key tiles
GKT = NKT // NG              # 8 key tiles per group

_CACHE = {}


def _round11(x):
    """Round fp32 to 11 explicit mantissa bits (fp32r grid), RNE."""
    u = np.ascontiguousarray(x, np.float32).view(np.uint32)
    bias = ((u >> 12) & 1) + np.uint32((1 << 11) - 1)
    u = (u + bias) & np.uint32(0xFFFFF000)
    return u.view(np.float32)


def _build_nc():
    import concourse.bacc as bacc
    import concourse.mybir as mybir
    from concourse.tile import TileContext

    f32 = mybir.dt.float32
    f32r = mybir.dt.float32r
    bf16 = mybir.dt.bfloat16
    EXP = mybir.ActivationFunctionType.Exp
    nc = bacc.Bacc("TRN2", target_bir_lowering=False, debug=False)

    qhh = nc.dram_tensor("qhh", [128, NQ], f32r, kind="ExternalInput")
    qp2 = nc.dram_tensor("qp2", [128, NQ], bf16, kind="ExternalInput")
    khi2 = nc.dram_tensor("khi2", [128, NTOK], f32r, kind="ExternalInput")
    kq2 = nc.dram_tensor("kq2", [128, NTOK], bf16, kind="ExternalInput")
    vpk = nc.dram_tensor("vpk", [128, NKT * 65], f32r, kind="ExternalInput")
    out = nc.dram_tensor("out", [65, NQ], f32, kind="ExternalOutput")

    GW = GKT * 128            # tokens per DMA group
    with TileContext(nc) as tc:
        with (
            tc.tile_pool(name="const", bufs=1) as const,
            tc.tile_pool(name="pp", bufs=4) as pp,
            tc.tile_pool(name="sbo", bufs=2) as sbo,
            tc.tile_pool(name="ps_s", bufs=3, space="PSUM") as ps_s,
            tc.tile_pool(name="ps_o", bufs=2, space="PSUM") as ps_o,
        ):
            # ---- PE + ACT warmup during the DMA prefix ----
            wz = const.tile([128, 512], bf16, tag="wz")
            nc.vector.memset(wz, 0.0)
            wexp = const.tile([128, 1], f32, tag="wexp")
            nc.scalar.activation(wexp, wz[:, 0:1], EXP)  # pull exp table load
            for _ in range(12):
                wps = ps_s.tile([128, 2 * CHW], f32, tag="s")
                nc.tensor.matmul(wps[:, 0:512], wz[:, 0:128], wz,
                                 start=True, stop=True)

            # ---- input DMAs (q first, then k-side in kt-groups) ----
            qhh_t = const.tile([128, NQ], f32r, tag="qhh")
            qp2_t = const.tile([128, NQ], bf16, tag="qp2")
            # chunk 0's q operands first so compute starts early
            nc.sync.dma_start(out=qhh_t[:, 0:CHW], in_=qhh[:, 0:CHW])
            nc.sync.dma_start(out=qp2_t[:, 0:CHW], in_=qp2[:, 0:CHW])
            khi2_g, kq2_g, vpk_g = [], [], []
            for g in range(NG):
                kt_ = const.tile([128, GW], f32r, tag=f"khi2_{g}")
                nc.sync.dma_start(out=kt_, in_=khi2[:, g * GW:(g + 1) * GW])
                khi2_g.append(kt_)
                kt_ = const.tile([128, GW], bf16, tag=f"kq2_{g}")
                nc.sync.dma_start(out=kt_, in_=kq2[:, g * GW:(g + 1) * GW])
                kq2_g.append(kt_)
                kt_ = const.tile([128, GKT * 65], f32r, tag=f"vpk_{g}")
                nc.sync.dma_start(
                    out=kt_, in_=vpk[:, g * GKT * 65:(g + 1) * GKT * 65])
                vpk_g.append(kt_)
                if g < NCH - 1:   # remaining q chunks, interleaved
                    cs = slice((g + 1) * CHW, (g + 2) * CHW)
                    nc.sync.dma_start(out=qhh_t[:, cs], in_=qhh[:, cs])
                    nc.sync.dma_start(out=qp2_t[:, cs], in_=qp2[:, cs])

            # ---- main loop ----
            for ch in range(NCH):
                qs = slice(ch * CHW, (ch + 1) * CHW)
                o_acc = ps_o.tile([65, CHW], f32, tag="oacc")
                for pr in range(NPAIR):
                    ktA, ktB = 2 * pr, 2 * pr + 1
                    g = ktA // GKT
                    lA = (ktA - g * GKT) * 128
                    lB = (ktB - g * GKT) * 128
                    s_t = ps_s.tile([128, 2 * CHW], f32, tag="s")
                    # pass1: k_hi . q_hi, fp32r, packed pair (rows 0-63 /
                    # 64-127) — B's weight load hides under A's stream
                    nc.tensor.matmul(
                        s_t[:, 0:CHW],
                        khi2_g[g][0:64, lA:lA + 128], qhh_t[0:64, qs],
                        start=True, stop=False,
                    )
                    nc.tensor.matmul(
                        s_t[:, CHW:2 * CHW],
                        khi2_g[g][64:128, lB:lB + 128], qhh_t[64:128, qs],
                        start=True, stop=False,
                    )
                    # pass2: cross terms + bias row, bf16, contraction 128
                    nc.tensor.matmul(
                        s_t[:, 0:CHW],
                        kq2_g[g][:, lA:lA + 128], qp2_t[:, qs],
                        start=False, stop=True,
                    )
                    nc.tensor.matmul(
                        s_t[:, CHW:2 * CHW],
                        kq2_g[g][:, lB:lB + 128], qp2_t[:, qs],
                        start=False, stop=True,
                    )
                    p_t = pp.tile([128, 2 * CHW], f32r, tag="p")
                    nc.scalar.activation(p_t, s_t, EXP)
                    for half, kt in ((0, ktA), (1, ktB)):
                        lv = (kt - g * GKT) * 65
                        nc.tensor.matmul(
                            o_acc[:, :],
                            vpk_g[g][:, lv:lv + 65],
                            p_t[:, half * CHW:(half + 1) * CHW],
                            start=(pr == 0 and half == 0),
                            stop=(pr == NPAIR - 1 and half == 1),
                            skip_group_check=True,
                        )
                # ---- ship O^T chunk (normalize + transpose on host) ----
                o_sb = sbo.tile([65, CHW], f32, tag="osb")
                nc.vector.tensor_copy(o_sb, o_acc)
                nc.sync.dma_start(out=out[:, qs], in_=o_sb)
    nc.compile()
    return nc


def _prep_inputs(x):
    """Host-side shard + operand marshaling. Returns list of 8 in_maps."""
    import ml_dtypes
    bf16 = ml_dtypes.bfloat16
    t = np.ascontiguousarray(x, np.float32).reshape(B_, NTOK, C_)
    in_maps = []
    for b in range(B_):
        kv = t[b]                                   # [4096, 64]
        k_hi = _round11(kv)
        k_lo = (kv - k_hi).astype(np.float32)
        kmax = float(np.linalg.norm(kv.astype(np.float64), axis=1).max())
        khi2 = np.concatenate([k_hi.T, k_hi.T]).astype(np.float32)
        kq2 = np.concatenate(
            [k_lo.T[0:63], np.ones((1, NTOK), np.float32), k_hi.T]
        ).astype(bf16)
        vpk = np.concatenate(
            [np.concatenate([kv[i * 128:(i + 1) * 128],
                             np.ones((128, 1), np.float32)], axis=1)
             for i in range(NKT)], axis=1).astype(np.float32)  # [128, 32*65]
        for h in range(2):
            q = t[b, h * NQ:(h + 1) * NQ]           # [2048, 64]
            q_hi = _round11(q)
            q_lo = (q - q_hi).astype(np.float32)
            qn = np.linalg.norm(q.astype(np.float64), axis=1)
            bias = (qn * kmax + 0.125).astype(np.float32)   # >= rowmax(s)
            qhh = np.concatenate([q_hi.T, q_hi.T]).astype(np.float32)
            qp2 = np.concatenate(
                [q_hi.T[0:63], -bias[None, :], q_lo.T]).astype(bf16)
            in_maps.append({
                "qhh": qhh, "qp2": qp2, "khi2": khi2, "kq2": kq2, "vpk": vpk,
            })
    return in_maps


def run(x, trace=False):
    from concourse.bass_utils import run_bass_kernel_spmd
    if "nc" not in _CACHE:
        _CACHE["nc"] = _build_nc()
    nc = _CACHE["nc"]
    in_maps = _prep_inputs(x)
    res = run_bass_kernel_spmd(
        nc, in_maps, core_ids=list(range(NCORES)), trace=trace,
    )
    full = np.empty((B_, NTOK, C_), np.float32)
    for b in range(B_):
        for h in range(2):
            o = res.results[2 * b + h]["out"]        # [65, 2048]
            full[b, h * NQ:(h + 1) * NQ] = (o[0:C_] / o[C_]).T
    return full.reshape(B_, D_, H_, W_, C_), res


def kernel(x):
    out, _ = run(x, trace=False)
    return out



# revision 5
# speedup vs baseline: 1.0927x; 1.0927x over previous
"""Trainium2 Bass kernel for nn_Denoiser_73598559584966.

Full-sequence self-attention (Q=K=V, no scaling) over x: [4, 16, 16, 16, 64]
  t = x.reshape(B, 4096, 64); out = softmax(t @ t^T) @ t

Sharding: 8 cores = 4 batches x 2 query-halves. Each core: 2048 queries
vs the full 4096 keys/values of its batch. No collectives.

Device algorithm per core (scores kept transposed: [keys, queries]),
key tiles in packed pairs (ktA rows 0-63 / ktB rows 64-127 of the PE
array so LDWEIGHTS hides under the other half's stream):
  S' = k . (A*q)      single-pass fp32r, contraction 64; A = 128/ln2 so
                      S' = A*s. Inputs pre-rounded to the fp32r grid.
  P  = exp(s - shift) as bf16, computed two ways, split by query column:
        ACT cols:  activation(Exp, scale=1/A, bias=-shift)   (exact exp)
        DVE cols:  bitcast_bf16(int16(max(S' + B', 0)))      (Schraudolph
                   fast exp on the bf16 grid; B' = 128*(127 - shift*log2 e)
                   - C). The ~2% relative error cancels in the softmax
                   ratio (softmax rows here are near-one-hot).
  O^T[65, q] += (V_kt|1)^T P_kt    bf16; row 64 = softmax denominator
shift is per query-chunk (>= rowmax - 30) so exp never overflows; it
cancels exactly in the softmax ratio. Host divides rows 0..63 by row 64
and transposes while gathering shards (O(N*C) marshaling).
"""
import numpy as np

B_, D_, H_, W_, C_ = 4, 16, 16, 16, 64
NTOK = D_ * H_ * W_          # 4096 tokens per batch
NQ = NTOK // 2               # 2048 queries per core
NCORES = 8
NKT = NTOK // 128            # 32 key tiles
NPAIR = NKT // 2             # 16 packed key-tile pairs
NCH = 4                      # query chunks per core
CHW = NQ // NCH              # 512 queries per chunk
NG = 4                       # DMA groups over key tiles
GKT = NKT // NG              # 8 key tiles per group

NA_H = 288                   # exp cols per half on ScalarE (exact exp)
ND_H = CHW - NA_H            # exp cols per half on VectorE (fast exp)
AEXP = 184.66350558899108    # 128 / ln 2  (bf16 Schraudolph scale)
C_SCH = 5.590103149414062    # Schraudolph bias-correction (bf16 grid)
MARGIN = 30.0                # shift = chunk score bound - MARGIN

_CACHE = {}


def _round11(x):
    """Round fp32 to 11 explicit mantissa bits (fp32r grid), RNE."""
    u = np.ascontiguousarray(x, np.float32).view(np.uint32)
    bias = ((u >> 12) & 1) + np.uint32((1 << 11) - 1)
    u = (u + bias) & np.uint32(0xFFFFF000)
    return u.view(np.float32)


def _build_nc():
    import concourse.bacc as bacc
    import concourse.mybir as mybir
    from concourse.tile import TileContext

    f32 = mybir.dt.float32
    f32r = mybir.dt.float32r
    i16 = mybir.dt.int16
    bf16 = mybir.dt.bfloat16
    EXP = mybir.ActivationFunctionType.Exp
    ADD = mybir.AluOpType.add
    MAX = mybir.AluOpType.max
    nc = bacc.Bacc("TRN2", target_bir_lowering=False, debug=False)

    qhh = nc.dram_tensor("qhh", [128, NQ], f32r, kind="ExternalInput")
    khi2 = nc.dram_tensor("khi2", [128, NTOK], f32r, kind="ExternalInput")
    vpk = nc.dram_tensor("vpk", [128, NKT * 65], bf16, kind="ExternalInput")
    bsh = nc.dram_tensor("bsh", [128, NCH], f32, kind="ExternalInput")
    bdv = nc.dram_tensor("bdv", [128, NCH], f32, kind="ExternalInput")
    out = nc.dram_tensor("out", [65, NQ], f32, kind="ExternalOutput")

    GW = GKT * 128            # tokens per DMA group
    with TileContext(nc) as tc:
        with (
            tc.tile_pool(name="const", bufs=1) as const,
            tc.tile_pool(name="pp", bufs=4) as pp,
            tc.tile_pool(name="sbo", bufs=2) as sbo,
            tc.tile_pool(name="ps_s", bufs=3, space="PSUM") as ps_s,
            tc.tile_pool(name="ps_o", bufs=2, space="PSUM") as ps_o,
        ):
            # ---- PE + ACT warmup during the DMA prefix ----
            wz = const.tile([128, 512], bf16, tag="wz")
            nc.vector.memset(wz, 0.0)
            wexp = const.tile([128, 1], f32, tag="wexp")
            nc.scalar.activation(wexp, wz[:, 0:1], EXP)  # pull exp table load
            for _ in range(12):
                wps = ps_s.tile([128, 2, CHW], f32, tag="s")
                nc.tensor.matmul(wps[:, 0, :], wz[:, 0:128], wz,
                                 start=True, stop=True)

            # ---- input DMAs (biases+chunk0 q first, then kt-groups) ----
            bsh_t = const.tile([128, NCH], f32, tag="bsh")
            nc.sync.dma_start(out=bsh_t, in_=bsh[:, :])
            bdv_t = const.tile([128, NCH], f32, tag="bdv")
            nc.sync.dma_start(out=bdv_t, in_=bdv[:, :])
            qhh_t = const.tile([128, NQ], f32r, tag="qhh")
            nc.sync.dma_start(out=qhh_t[:, 0:CHW], in_=qhh[:, 0:CHW])
            khi2_g, vpk_g = [], []
            for g in range(NG):
                kt_ = const.tile([128, GW], f32r, tag=f"khi2_{g}")
                nc.sync.dma_start(out=kt_, in_=khi2[:, g * GW:(g + 1) * GW])
                khi2_g.append(kt_)
                kt_ = const.tile([128, GKT * 65], bf16, tag=f"vpk_{g}")
                nc.sync.dma_start(
                    out=kt_, in_=vpk[:, g * GKT * 65:(g + 1) * GKT * 65])
                vpk_g.append(kt_)
                if g < NCH - 1:   # remaining q chunks, interleaved
                    cs = slice((g + 1) * CHW, (g + 2) * CHW)
                    nc.sync.dma_start(out=qhh_t[:, cs], in_=qhh[:, cs])

            # ---- main loop (PV of pair p issued after scores of p+1) ----
            for ch in range(NCH):
                qs = slice(ch * CHW, (ch + 1) * CHW)
                o_acc = ps_o.tile([65, CHW], f32, tag="oacc")
                prev_p = None

                def pv(pr, p_t):
                    g = (2 * pr) // GKT
                    for half in range(2):
                        kt = 2 * pr + half
                        lv = (kt - g * GKT) * 65
                        nc.tensor.matmul(
                            o_acc[:, :],
                            vpk_g[g][:, lv:lv + 65],
                            p_t[:, half, :],
                            start=(pr == 0 and half == 0),
                            stop=(pr == NPAIR - 1 and half == 1),
                            skip_group_check=True,
                        )

                for pr in range(NPAIR):
                    ktA, ktB = 2 * pr, 2 * pr + 1
                    g = ktA // GKT
                    lA = (ktA - g * GKT) * 128
                    lB = (ktB - g * GKT) * 128
                    s_t = ps_s.tile([128, 2, CHW], f32, tag="s")
                    # packed pair: ktA on PE rows 0-63, ktB on rows 64-127
                    nc.tensor.matmul(
                        s_t[:, 0, :],
                        khi2_g[g][0:64, lA:lA + 128], qhh_t[0:64, qs],
                        start=True, stop=True, skip_group_check=True,
                    )
                    nc.tensor.matmul(
                        s_t[:, 1, :],
                        khi2_g[g][64:128, lB:lB + 128], qhh_t[64:128, qs],
                        start=True, stop=True, skip_group_check=True,
                    )
                    p_t = pp.tile([128, 2, CHW], bf16, tag="p")
                    # exact exp on ScalarE for the head columns
                    nc.scalar.activation(
                        p_t[:, :, 0:NA_H], s_t[:, :, 0:NA_H], EXP,
                        bias=bsh_t[:, ch:ch + 1], scale=1.0 / AEXP)
                    if ND_H:
                        # Schraudolph fast exp on VectorE for the tail
                        nc.vector.tensor_scalar(
                            p_t[:, :, NA_H:].bitcast(i16),
                            s_t[:, :, NA_H:],
                            bdv_t[:, ch:ch + 1], 0.0, ADD, MAX)
                    if prev_p is not None:
                        pv(pr - 1, prev_p)
                    prev_p = p_t
                pv(NPAIR - 1, prev_p)
                # ---- ship O^T chunk (normalize + transpose on host) ----
                o_sb = sbo.tile([65, CHW], f32, tag="osb")
                nc.vector.tensor_copy(o_sb, o_acc)
                nc.sync.dma_start(out=out[:, qs], in_=o_sb)
    nc.compile()
    return nc


def _prep_inputs(x):
    """Host-side shard + operand marshaling. Returns list of 8 in_maps."""
    import ml_dtypes
    bf16 = ml_dtypes.bfloat16
    t = np.ascontiguousarray(x, np.float32).reshape(B_, NTOK, C_)
    in_maps = []
    for b in range(B_):
        kv = t[b]                                   # [4096, 64]
        k_hi = _round11(kv)
        kmax = float(np.linalg.norm(kv.astype(np.float64), axis=1).max())
        khi2 = np.concatenate([k_hi.T, k_hi.T]).astype(np.float32)
        vpk = np.concatenate(
            [np.concatenate([kv[i * 128:(i + 1) * 128],
                             np.ones((128, 1), np.float32)], axis=1)
             for i in range(NKT)], axis=1).astype(bf16)  # [128, 32*65]
        for h in range(2):
            q = t[b, h * NQ:(h + 1) * NQ]           # [2048, 64]
            q_hi = _round11(q)
            qa = _round11((q_hi * np.float32(AEXP)).astype(np.float32))
            qhh = np.concatenate([qa.T, qa.T]).astype(np.float32)
            shift = np.empty(NCH, np.float64)
            for c in range(NCH):
                qn = np.linalg.norm(
                    q[c * CHW:(c + 1) * CHW].astype(np.float64), axis=1).max()
                shift[c] = qn * kmax - MARGIN
            bsh = np.broadcast_to(
                (-shift).astype(np.float32), (128, NCH)).copy()
            bdv = np.broadcast_to(
                (16256.0 - C_SCH - AEXP * shift).astype(np.float32),
                (128, NCH)).copy()
            in_maps.append({
                "qhh": qhh, "khi2": khi2, "vpk": vpk, "bsh": bsh, "bdv": bdv,
            })
    return in_maps


def run(x, trace=False):
    from concourse.bass_utils import run_bass_kernel_spmd
    if "nc" not in _CACHE:
        _CACHE["nc"] = _build_nc()
    nc = _CACHE["nc"]
    in_maps = _prep_inputs(x)
    res = run_bass_kernel_spmd(
        nc, in_maps, core_ids=list(range(NCORES)), trace=trace,
    )
    full = np.empty((B_, NTOK, C_), np.float32)
    for b in range(B_):
        for h in range(2):
            o = res.results[2 * b + h]["out"]        # [65, 2048]
            full[b, h * NQ:(h + 1) * NQ] = (o[0:C_] / o[C_]).T
    return full.reshape(B_, D_, H_, W_, C_), res


def kernel(x):
    out, _ = run(x, trace=False)
    return out


# revision 6
# speedup vs baseline: 1.3872x; 1.2695x over previous
"""Trainium2 Bass kernel for nn_Denoiser_73598559584966.

Full-sequence self-attention (Q=K=V, no scaling) over x: [4, 16, 16, 16, 64]
  t = x.reshape(B, 4096, 64); out = softmax(t @ t^T) @ t

Sharding: 8 cores = 4 batches x 2 query-halves. Each core: 2048 queries
vs the full 4096 keys/values of its batch. No collectives.

Device algorithm per core (scores kept transposed: [keys, queries]),
key tiles in packed pairs (ktA rows 0-63 / ktB rows 64-127 of the PE
array so LDWEIGHTS hides under the other half's stream); all matmul
operands bf16 so FWL (fast weight load) can engage:
  S' = k . (A*q)      single-pass bf16, contraction 64; A = 128/ln2 so
                      S' = A*s accumulates fp32 in PSUM.
  P  = exp(s - shift) as bf16, computed two ways, split by query column:
        ACT cols:  activation(Exp, scale=1/A, bias=-shift)   (exact exp)
        DVE cols:  bitcast_bf16(int16(max(S' + B', 0)))      (Schraudolph
                   fast exp on the bf16 grid; B' = 128*(127 - shift*log2 e)
                   - C). The ~2% relative error cancels in the softmax
                   ratio (softmax rows here are near-one-hot).
  O^T[128, q] += Vp_kt^T P_kt   bf16 weights [128 keys, 128]: cols 0-63 =
                   V, col 64 = ones (softmax denominator), rest zero pad
                   so the weight load is FWL-eligible and hides.
shift is per query-chunk (>= rowmax - 30) so exp never overflows; it
cancels exactly in the softmax ratio. Host divides rows 0..63 by row 64
and transposes while gathering shards (O(N*C) marshaling).

Schedule notes: PV of pair p issues after the scores of pair p+1 so the
PE never waits on the exp; the first key-tile group and query chunk are
DMA'd first with PE warmup matmuls covering the wait (a >3.4us PE idle
gap would HAM-rethrottle the PE to 1.2 GHz, and that throttle has been
seen sticking for ~40us).
"""
import numpy as np

B_, D_, H_, W_, C_ = 4, 16, 16, 16, 64
NTOK = D_ * H_ * W_          # 4096 tokens per batch
NQ = NTOK // 2               # 2048 queries per core
NCORES = 8
NKT = NTOK // 128            # 32 key tiles
NPAIR = NKT // 2             # 16 packed key-tile pairs
NCH = 4                      # query chunks per core
CHW = NQ // NCH              # 512 queries per chunk
NG = 4                       # DMA groups over key tiles
GKT = NKT // NG              # 8 key tiles per group

NA_H = 288                   # exp cols per half on ScalarE (exact exp)
ND_H = CHW - NA_H            # exp cols per half on VectorE (fast exp)
AEXP = 184.66350558899108    # 128 / ln 2  (bf16 Schraudolph scale)
C_SCH = 5.590103149414062    # Schraudolph bias-correction (bf16 grid)
MARGIN = 30.0                # shift = chunk score bound - MARGIN
NWARM = 12                   # PE warmup matmuls under the DMA prefix

_CACHE = {}


def _build_nc():
    import concourse.bacc as bacc
    import concourse.mybir as mybir
    from concourse.tile import TileContext

    f32 = mybir.dt.float32
    i16 = mybir.dt.int16
    bf16 = mybir.dt.bfloat16
    EXP = mybir.ActivationFunctionType.Exp
    ADD = mybir.AluOpType.add
    MAX = mybir.AluOpType.max
    nc = bacc.Bacc("TRN2", target_bir_lowering=False, debug=False)

    qhh = nc.dram_tensor("qhh", [128, NQ], bf16, kind="ExternalInput")
    khi2 = nc.dram_tensor("khi2", [128, NTOK], bf16, kind="ExternalInput")
    vpk = nc.dram_tensor("vpk", [128, NKT * 128], bf16, kind="ExternalInput")
    bsh = nc.dram_tensor("bsh", [128, NCH], f32, kind="ExternalInput")
    bdv = nc.dram_tensor("bdv", [128, NCH], f32, kind="ExternalInput")
    out = nc.dram_tensor("out", [65, NQ], f32, kind="ExternalOutput")

    GW = GKT * 128            # tokens per DMA group
    with TileContext(nc) as tc:
        with (
            tc.tile_pool(name="const", bufs=1) as const,
            tc.tile_pool(name="pp", bufs=4) as pp,
            tc.tile_pool(name="sbo", bufs=2) as sbo,
            tc.tile_pool(name="ps_s", bufs=3, space="PSUM") as ps_s,
            tc.tile_pool(name="ps_o", bufs=2, space="PSUM") as ps_o,
        ):
            # ---- PE + ACT warmup during the DMA prefix ----
            wz = const.tile([128, 512], bf16, tag="wz")
            nc.vector.memset(wz, 0.0)
            wexp = const.tile([128, 1], f32, tag="wexp")
            nc.scalar.activation(wexp, wz[:, 0:1], EXP)  # pull exp table load
            for _ in range(NWARM):
                wps = ps_s.tile([128, 2, CHW], f32, tag="s")
                nc.tensor.matmul(wps[:, 0, :], wz[:, 0:128], wz,
                                 start=True, stop=True)

            # ---- input DMAs: first-chunk operands first, then the rest ----
            qhh_t = const.tile([128, NQ], bf16, tag="qhh")
            nc.sync.dma_start(out=qhh_t[:, 0:CHW], in_=qhh[:, 0:CHW])
            khi2_g, vpk_g = [], []
            for g in range(NG):
                kt_ = const.tile([128, GW], bf16, tag=f"khi2_{g}")
                khi2_g.append(kt_)
                kt_ = const.tile([128, GKT * 128], bf16, tag=f"vpk_{g}")
                vpk_g.append(kt_)
            nc.sync.dma_start(out=khi2_g[0], in_=khi2[:, 0:GW])
            bsh_t = const.tile([128, NCH], f32, tag="bsh")
            nc.sync.dma_start(out=bsh_t, in_=bsh[:, :])
            bdv_t = const.tile([128, NCH], f32, tag="bdv")
            nc.sync.dma_start(out=bdv_t, in_=bdv[:, :])
            nc.sync.dma_start(out=vpk_g[0], in_=vpk[:, 0:GKT * 128])
            for g in range(1, NG):
                nc.sync.dma_start(
                    out=khi2_g[g], in_=khi2[:, g * GW:(g + 1) * GW])
                nc.sync.dma_start(
                    out=vpk_g[g],
                    in_=vpk[:, g * GKT * 128:(g + 1) * GKT * 128])
                cs = slice(g * CHW, (g + 1) * CHW)
                nc.sync.dma_start(out=qhh_t[:, cs], in_=qhh[:, cs])

            # ---- main loop (PV of pair p issued after scores of p+1) ----
            for ch in range(NCH):
                qs = slice(ch * CHW, (ch + 1) * CHW)
                o_acc = ps_o.tile([128, CHW], f32, tag="oacc")
                prev_p = None

                def pv(pr, p_t):
                    g = (2 * pr) // GKT
                    for half in range(2):
                        kt = 2 * pr + half
                        lv = (kt - g * GKT) * 128
                        nc.tensor.matmul(
                            o_acc[:, :],
                            vpk_g[g][:, lv:lv + 128],
                            p_t[:, half, :],
                            start=(pr == 0 and half == 0),
                            stop=(pr == NPAIR - 1 and half == 1),
                            skip_group_check=True,
                        )

                for pr in range(NPAIR):
                    ktA, ktB = 2 * pr, 2 * pr + 1
                    g = ktA // GKT
                    lA = (ktA - g * GKT) * 128
                    lB = (ktB - g * GKT) * 128
                    s_t = ps_s.tile([128, 2, CHW], f32, tag="s")
                    # packed pair: ktA on PE rows 0-63, ktB on rows 64-127
                    nc.tensor.matmul(
                        s_t[:, 0, :],
                        khi2_g[g][0:64, lA:lA + 128], qhh_t[0:64, qs],
                        start=True, stop=True, skip_group_check=True,
                    )
                    nc.tensor.matmul(
                        s_t[:, 1, :],
                        khi2_g[g][64:128, lB:lB + 128], qhh_t[64:128, qs],
                        start=True, stop=True, skip_group_check=True,
                    )
                    p_t = pp.tile([128, 2, CHW], bf16, tag="p")
                    # exact exp on ScalarE for the head columns
                    nc.scalar.activation(
                        p_t[:, :, 0:NA_H], s_t[:, :, 0:NA_H], EXP,
                        bias=bsh_t[:, ch:ch + 1], scale=1.0 / AEXP)
                    if ND_H:
                        # Schraudolph fast exp on VectorE for the tail
                        nc.vector.tensor_scalar(
                            p_t[:, :, NA_H:].bitcast(i16),
                            s_t[:, :, NA_H:],
                            bdv_t[:, ch:ch + 1], 0.0, ADD, MAX)
                    if prev_p is not None:
                        pv(pr - 1, prev_p)
                    prev_p = p_t
                pv(NPAIR - 1, prev_p)
                # ---- ship O^T chunk (normalize + transpose on host) ----
                o_sb = sbo.tile([65, CHW], f32, tag="osb")
                nc.vector.tensor_copy(o_sb, o_acc[0:65, :])
                nc.sync.dma_start(out=out[:, qs], in_=o_sb)
    nc.compile()
    return nc


def _prep_inputs(x):
    """Host-side shard + operand marshaling. Returns list of 8 in_maps."""
    import ml_dtypes
    bf16 = ml_dtypes.bfloat16
    t = np.ascontiguousarray(x, np.float32).reshape(B_, NTOK, C_)
    in_maps = []
    for b in range(B_):
        kv = t[b]                                   # [4096, 64]
        k_hi = kv.astype(bf16)
        kmax = float(np.linalg.norm(kv.astype(np.float64), axis=1).max())
        khi2 = np.concatenate([k_hi.T, k_hi.T])     # [128, 4096] bf16
        vcols = np.zeros((NTOK, 128), np.float32)
        vcols[:, 0:C_] = kv
        vcols[:, C_] = 1.0
        vpk = np.concatenate(
            [vcols[i * 128:(i + 1) * 128] for i in range(NKT)],
            axis=1).astype(bf16)                    # [128, 32*128]
        for h in range(2):
            q = t[b, h * NQ:(h + 1) * NQ]           # [2048, 64]
            qa = (q.astype(bf16).astype(np.float32)
                  * np.float32(AEXP)).astype(bf16)
            qhh = np.concatenate([qa.T, qa.T])      # [128, 2048] bf16
            shift = np.empty(NCH, np.float64)
            for c in range(NCH):
                qn = np.linalg.norm(
                    q[c * CHW:(c + 1) * CHW].astype(np.float64), axis=1).max()
                shift[c] = qn * kmax - MARGIN
            bsh = np.broadcast_to(
                (-shift).astype(np.float32), (128, NCH)).copy()
            bdv = np.broadcast_to(
                (16256.0 - C_SCH - AEXP * shift).astype(np.float32),
                (128, NCH)).copy()
            in_maps.append({
                "qhh": qhh, "khi2": khi2, "vpk": vpk, "bsh": bsh, "bdv": bdv,
            })
    return in_maps


def run(x, trace=False):
    from concourse.bass_utils import run_bass_kernel_spmd
    if "nc" not in _CACHE:
        _CACHE["nc"] = _build_nc()
    nc = _CACHE["nc"]
    in_maps = _prep_inputs(x)
    res = run_bass_kernel_spmd(
        nc, in_maps, core_ids=list(range(NCORES)), trace=trace,
    )
    full = np.empty((B_, NTOK, C_), np.float32)
    for b in range(B_):
        for h in range(2):
            o = res.results[2 * b + h]["out"]        # [65, 2048]
            full[b, h * NQ:(h + 1) * NQ] = (o[0:C_] / o[C_]).T
    return full.reshape(B_, D_, H_, W_, C_), res


def kernel(x):
    out, _ = run(x, trace=False)
    return out
